# revision 8
# baseline (speedup 1.0000x reference)
import os
import sys
from contextlib import ExitStack

import numpy as np
import ml_dtypes

for _p in ("/opt/trn_rl_repo", "/root/.axon_site/_ro/trn_rl_repo"):
    if os.path.isdir(_p) and _p not in sys.path:
        sys.path.append(_p)

DEPTH = 13
B = 16
X = 256
H = 128
A = 2
N = 2 ** (DEPTH + 1) - 1          # 16383 nodes per tree
NCORES = 8
TPC = B // NCORES                  # trees per core = 2
NPC = TPC * N                      # nodes per core = 32766
FMAX = 512                         # node columns per chunk

BF16 = ml_dtypes.bfloat16

_cached = None
RUN_KW = {}
LAST = None
LAST_IN_MAPS = None


def _build():
    import concourse.bacc as bacc
    import concourse.tile as tile
    from concourse import mybir

    f32 = mybir.dt.float32
    bf16 = mybir.dt.bfloat16
    Alu = mybir.AluOpType
    Act = mybir.ActivationFunctionType

    nc = bacc.Bacc(None)
    xt = nc.declare_dram_parameter("xt", [X, NPC], bf16, isOutput=False)
    wwt = nc.declare_dram_parameter("wwt", [2, 128, 512], bf16, isOutput=False)
    urt = nc.declare_dram_parameter("urt", [2, 128, 128], bf16, isOutput=False)
    uht = nc.declare_dram_parameter("uht", [2, 128, 128], bf16, isOutput=False)
    uzt = nc.declare_dram_parameter("uzt", [2, 128, 256], bf16, isOutput=False)
    bias = nc.declare_dram_parameter("bias", [128, 4], f32, isOutput=False)
    h_out = nc.declare_dram_parameter("h_out", [H, NPC], bf16, isOutput=True)

    CH = 1024                       # columns per processing chunk (ACT/DVE width)
    SLAB = 4096                     # x slab columns per DMA

    with tile.TileContext(nc) as tc, ExitStack() as ctx:
        const = ctx.enter_context(tc.tile_pool(name="const", bufs=1))
        hpool = ctx.enter_context(tc.tile_pool(name="hbuf", bufs=1))
        xpool = ctx.enter_context(tc.tile_pool(name="xin", bufs=3))
        spool = ctx.enter_context(tc.tile_pool(name="inter", bufs=2))
        ppool = ctx.enter_context(tc.tile_pool(name="psum", bufs=1, space="PSUM"))

        w_ww = []
        for k in range(2):
            t = const.tile([128, 512], bf16, tag=f"ww{k}", name=f"w_ww{k}")
            nc.sync.dma_start(t[:], wwt[k])
            w_ww.append(t)
        w_ur = []
        w_uh = []
        w_uz = []
        for k in range(2):
            t = const.tile([128, 128], bf16, tag=f"ur{k}", name=f"w_ur{k}")
            nc.sync.dma_start(t[:], urt[k])
            w_ur.append(t)
            t = const.tile([128, 128], bf16, tag=f"uh{k}", name=f"w_uh{k}")
            nc.sync.dma_start(t[:], uht[k])
            w_uh.append(t)
            t = const.tile([128, 256], bf16, tag=f"uz{k}", name=f"w_uz{k}")
            nc.sync.dma_start(t[:], uzt[k])
            w_uz.append(t)
        bias_s = const.tile([128, 4], f32, tag="bias", name="bias_s")
        nc.sync.dma_start(bias_s[:], bias[:])
        b_r = bias_s[:, 0:1]
        b_z0 = bias_s[:, 1:2]
        b_z1 = bias_s[:, 2:3]
        b_hcn = bias_s[:, 3:4]

        # h ping-pong buffers per tree, stored as (parent, side) pairs:
        # buffer for level l (parity l%2) holds h of level-l nodes; node j of
        # level l sits at [:, j>>1, j&1] so level l-1 reads children without
        # any gather.
        hb = [
            [
                hpool.tile([128, 2048, 2], bf16, tag=f"h{t}0", name=f"hb{t}0"),
                hpool.tile([128, 4096, 2], bf16, tag=f"h{t}1", name=f"hb{t}1"),
            ]
            for t in range(TPC)
        ]

        for lvl in range(DEPTH, -1, -1):
            Fl = 2 ** lvl
            leaf = lvl == DEPTH
            par = lvl % 2
            for t in range(TPC):
                base = t * N + (Fl - 1)
                xs = []
                for s0 in range(0, Fl, SLAB):
                    W = min(SLAB, Fl - s0)
                    xt0 = xpool.tile([128, W], bf16, tag="x0", name="x0")
                    nc.sync.dma_start(xt0[:], xt[0:128, base + s0:base + s0 + W])
                    xt1 = xpool.tile([128, W], bf16, tag="x1", name="x1")
                    nc.sync.dma_start(xt1[:], xt[128:256, base + s0:base + s0 + W])
                    xs.append((xt0, xt1))

                hsrc = None if leaf else hb[t][(lvl + 1) % 2]

                for j0 in range(0, Fl, CH):
                    C = min(CH, Fl - j0)
                    xk = xs[j0 // SLAB]
                    xo = j0 % SLAB
                    halves = [(q * 512, min(512, C - q * 512))
                              for q in range((C + 511) // 512)]

                    def xmov(k, qo, qw):
                        return xk[k][:, xo + qo:xo + qo + qw]

                    def hmov(side, qo, qw):
                        return hsrc[:, j0 + qo:j0 + qo + qw, side]

                    def accum(ps, contribs, open_=True, close=True):
                        for ci, (w, mov) in enumerate(contribs):
                            for qo, qw in halves:
                                nc.tensor.matmul(
                                    ps[:, qo:qo + qw], w, mov(qo, qw),
                                    start=(open_ and ci == 0),
                                    stop=(close and ci == len(contribs) - 1))

                    if not leaf:
                        ps_r = ppool.tile([128, C], f32, tag="ps_r", name="ps_r")
                        accum(ps_r, [
                            (w_ww[0][:, 0:128], lambda qo, qw: xmov(0, qo, qw)),
                            (w_ww[1][:, 0:128], lambda qo, qw: xmov(1, qo, qw)),
                            (w_ur[0][:], lambda qo, qw: hmov(0, qo, qw)),
                            (w_ur[1][:], lambda qo, qw: hmov(1, qo, qw)),
                        ])
                        ps_z0 = ppool.tile([128, C], f32, tag="ps_z0", name="ps_z0")
                        accum(ps_z0, [
                            (w_ww[0][:, 256:384], lambda qo, qw: xmov(0, qo, qw)),
                            (w_ww[1][:, 256:384], lambda qo, qw: xmov(1, qo, qw)),
                            (w_uz[0][:, 0:128], lambda qo, qw: hmov(0, qo, qw)),
                            (w_uz[1][:, 0:128], lambda qo, qw: hmov(1, qo, qw)),
                        ])
                        ps_z1 = ppool.tile([128, C], f32, tag="ps_z1", name="ps_z1")
                        accum(ps_z1, [
                            (w_ww[0][:, 384:512], lambda qo, qw: xmov(0, qo, qw)),
                            (w_ww[1][:, 384:512], lambda qo, qw: xmov(1, qo, qw)),
                            (w_uz[0][:, 128:256], lambda qo, qw: hmov(0, qo, qw)),
                            (w_uz[1][:, 128:256], lambda qo, qw: hmov(1, qo, qw)),
                        ])
                        ps_hc = ppool.tile([128, C], f32, tag="ps_hc", name="ps_hc")
                        accum(ps_hc, [
                            (w_ww[0][:, 128:256], lambda qo, qw: xmov(0, qo, qw)),
                            (w_ww[1][:, 128:256], lambda qo, qw: xmov(1, qo, qw)),
                        ], close=False)

                        r = spool.tile([128, C], bf16, tag="r", name="r")
                        nc.scalar.activation(r[:], ps_r[:], Act.Sigmoid, bias=b_r)
                        rh0 = spool.tile([128, C], bf16, tag="rh0", name="rh0")
                        nc.vector.tensor_mul(rh0[:], r[:], hmov(0, 0, C))
                        rh1 = spool.tile([128, C], bf16, tag="rh1", name="rh1")
                        nc.vector.tensor_mul(rh1[:], r[:], hmov(1, 0, C))
                        accum(ps_hc, [
                            (w_uh[0][:], lambda qo, qw: rh0[:, qo:qo + qw]),
                            (w_uh[1][:], lambda qo, qw: rh1[:, qo:qo + qw]),
                        ], open_=False)

                        z0 = spool.tile([128, C], bf16, tag="z0", name="z0")
                        nc.scalar.activation(z0[:], ps_z0[:], Act.Sigmoid, bias=b_z0)
                        z1 = spool.tile([128, C], bf16, tag="z1", name="z1")
                        nc.scalar.activation(z1[:], ps_z1[:], Act.Sigmoid, bias=b_z1)
                        # hcn = tanh(-(ps_hc + b_hc)) = -hcand
                        hcn = spool.tile([128, C], bf16, tag="hcn", name="hcn")
                        nc.scalar.activation(hcn[:], ps_hc[:], Act.Tanh,
                                             bias=b_hcn, scale=-1.0)

                        a = spool.tile([128, C], bf16, tag="a", name="a")
                        nc.gpsimd.tensor_mul(a[:], z0[:], hmov(0, 0, C))
                        bb = spool.tile([128, C], bf16, tag="bb", name="bb")
                        nc.gpsimd.tensor_mul(bb[:], z1[:], hmov(1, 0, C))
                        c = spool.tile([128, C], bf16, tag="c", name="c")
                        nc.gpsimd.tensor_add(c[:], a[:], bb[:])
                        s = spool.tile([128, C], bf16, tag="s", name="s")
                        nc.vector.tensor_add(s[:], z0[:], z1[:])
                        # p = (s - 1) * (-hcand) = (1 - s) * hcand
                        p = spool.tile([128, C], bf16, tag="p", name="p")
                        nc.vector.scalar_tensor_tensor(
                            p[:], s[:], 1.0, hcn[:], Alu.subtract, Alu.mult)
                    else:
                        ps_z0 = ppool.tile([128, C], f32, tag="ps_z0", name="ps_z0")
                        accum(ps_z0, [
                            (w_ww[0][:, 256:384], lambda qo, qw: xmov(0, qo, qw)),
                            (w_ww[1][:, 256:384], lambda qo, qw: xmov(1, qo, qw)),
                        ])
                        ps_z1 = ppool.tile([128, C], f32, tag="ps_z1", name="ps_z1")
                        accum(ps_z1, [
                            (w_ww[0][:, 384:512], lambda qo, qw: xmov(0, qo, qw)),
                            (w_ww[1][:, 384:512], lambda qo, qw: xmov(1, qo, qw)),
                        ])
                        ps_hc = ppool.tile([128, C], f32, tag="ps_hc", name="ps_hc")
                        accum(ps_hc, [
                            (w_ww[0][:, 128:256], lambda qo, qw: xmov(0, qo, qw)),
                            (w_ww[1][:, 128:256], lambda qo, qw: xmov(1, qo, qw)),
                        ])
                        z0 = spool.tile([128, C], bf16, tag="z0", name="z0")
                        nc.scalar.activation(z0[:], ps_z0[:], Act.Sigmoid, bias=b_z0)
                        z1 = spool.tile([128, C], bf16, tag="z1", name="z1")
                        nc.scalar.activation(z1[:], ps_z1[:], Act.Sigmoid, bias=b_z1)
                        hcn = spool.tile([128, C], bf16, tag="hcn", name="hcn")
                        nc.scalar.activation(hcn[:], ps_hc[:], Act.Tanh,
                                             bias=b_hcn, scale=-1.0)
                        s = spool.tile([128, C], bf16, tag="s", name="s")
                        nc.vector.tensor_add(s[:], z0[:], z1[:])

                    if lvl > 0:
                        dst = hb[t][par][:, j0 // 2:(j0 + C) // 2, :]
                    else:
                        rt = spool.tile([128, 1], bf16, tag="root", name="rt")
                        dst = rt[:]
                    if not leaf:
                        nc.vector.tensor_add(dst, c[:], p[:])
                    else:
                        nc.gpsimd.scalar_tensor_tensor(
                            dst, s[:], 1.0, hcn[:], Alu.subtract, Alu.mult)

                if lvl > 0:
                    nc.sync.dma_start(h_out[:, base:base + Fl],
                                      hb[t][par][:, 0:Fl // 2, :])
                else:
                    nc.sync.dma_start(h_out[:, base:base + 1], rt[:])

    nc.finalize()
    return nc


def _get_nc():
    global _cached
    if _cached is None:
        _cached = _build()
    return _cached


def kernel(**inputs):
    x = np.asarray(inputs["x"], dtype=np.float32)
    W_w = np.asarray(inputs["W_w"], dtype=np.float32)
    W_b = np.asarray(inputs["W_b"], dtype=np.float32)
    U_r = np.asarray(inputs["U_r"], dtype=np.float32)
    U_h = np.asarray(inputs["U_h"], dtype=np.float32)
    U_z = np.asarray(inputs["U_z"], dtype=np.float32)

    from concourse.bass_utils import run_bass_kernel_spmd

    nc = _get_nc()

    xb = x.astype(BF16)
    wwt = np.ascontiguousarray(W_w.T).reshape(2, 128, 512).astype(BF16)
    urt = np.ascontiguousarray(U_r.T).reshape(2, 128, 128).astype(BF16)
    uht = np.ascontiguousarray(U_h.T).reshape(2, 128, 128).astype(BF16)
    uzt = np.ascontiguousarray(U_z.T).reshape(2, 128, 256).astype(BF16)
    bias = np.stack(
        [W_b[:128], W_b[256:384], W_b[384:512], -W_b[128:256]], axis=1
    ).astype(np.float32)

    in_maps = []
    for c in range(NCORES):
        xt_c = np.ascontiguousarray(xb[c * NPC:(c + 1) * NPC].T)
        in_maps.append({
            "xt": xt_c, "wwt": wwt, "urt": urt, "uht": uht,
            "uzt": uzt, "bias": bias,
        })

    res = run_bass_kernel_spmd(nc, in_maps, list(range(NCORES)), **RUN_KW)
    global LAST, LAST_IN_MAPS
    LAST = res
    LAST_IN_MAPS = in_maps
    h = np.concatenate(
        [np.asarray(r["h_out"]).T for r in res.results], axis=0
    ).astype(np.float32)
    return h


# revision 10
# speedup vs baseline: 1.3300x; 1.3300x over previous
import os
import sys
from contextlib import ExitStack

import numpy as np
import ml_dtypes

for _p in ("/opt/trn_rl_repo", "/root/.axon_site/_ro/trn_rl_repo"):
    if os.path.isdir(_p) and _p not in sys.path:
        sys.path.append(_p)

DEPTH = 13
B = 16
X = 256
H = 128
A = 2
N = 2 ** (DEPTH + 1) - 1          # 16383 nodes per tree
NCORES = 8
TPC = B // NCORES                  # trees per core = 2
NPC = TPC * N                      # nodes per core = 32766
FMAX = 512                         # node columns per chunk

BF16 = ml_dtypes.bfloat16

_cached = None
RUN_KW = {}
LAST = None
LAST_IN_MAPS = None


def _build():
    import concourse.bacc as bacc
    import concourse.tile as tile
    from concourse import mybir

    f32 = mybir.dt.float32
    bf16 = mybir.dt.bfloat16
    Alu = mybir.AluOpType
    Act = mybir.ActivationFunctionType

    nc = bacc.Bacc(None)
    xt = nc.declare_dram_parameter("xt", [X, NPC], bf16, isOutput=False)
    wwt = nc.declare_dram_parameter("wwt", [2, 128, 512], bf16, isOutput=False)
    urt = nc.declare_dram_parameter("urt", [2, 128, 128], bf16, isOutput=False)
    uht = nc.declare_dram_parameter("uht", [2, 128, 128], bf16, isOutput=False)
    uzt = nc.declare_dram_parameter("uzt", [2, 128, 256], bf16, isOutput=False)
    bias = nc.declare_dram_parameter("bias", [128, 4], f32, isOutput=False)
    h_out = nc.declare_dram_parameter("h_out", [H, NPC], bf16, isOutput=True)

    CH = 1024                       # columns per processing chunk (ACT/DVE width)
    SLAB = 4096                     # x slab columns per DMA

    with tile.TileContext(nc) as tc, ExitStack() as ctx:
        const = ctx.enter_context(tc.tile_pool(name="const", bufs=1))
        hpool = ctx.enter_context(tc.tile_pool(name="hbuf", bufs=1))
        xpool = ctx.enter_context(tc.tile_pool(name="xin", bufs=3))
        spool = ctx.enter_context(tc.tile_pool(name="inter", bufs=2))
        ppool = ctx.enter_context(tc.tile_pool(name="psum", bufs=1, space="PSUM"))

        w_ww = []
        for k in range(2):
            t = const.tile([128, 512], bf16, tag=f"ww{k}", name=f"w_ww{k}")
            nc.sync.dma_start(t[:], wwt[k])
            w_ww.append(t)
        w_ur = []
        w_uh = []
        w_uz = []
        for k in range(2):
            t = const.tile([128, 128], bf16, tag=f"ur{k}", name=f"w_ur{k}")
            nc.sync.dma_start(t[:], urt[k])
            w_ur.append(t)
            t = const.tile([128, 128], bf16, tag=f"uh{k}", name=f"w_uh{k}")
            nc.sync.dma_start(t[:], uht[k])
            w_uh.append(t)
            t = const.tile([128, 256], bf16, tag=f"uz{k}", name=f"w_uz{k}")
            nc.sync.dma_start(t[:], uzt[k])
            w_uz.append(t)
        bias_s = const.tile([128, 4], f32, tag="bias", name="bias_s")
        nc.sync.dma_start(bias_s[:], bias[:])
        b_r = bias_s[:, 0:1]
        b_z0 = bias_s[:, 1:2]
        b_z1 = bias_s[:, 2:3]
        b_hcn = bias_s[:, 3:4]

        # h ping-pong buffers per tree, stored as (parent, side) pairs:
        # buffer for level l (parity l%2) holds h of level-l nodes; node j of
        # level l sits at [:, j>>1, j&1] so level l-1 reads children without
        # any gather.  Levels <= SMALL_MAX merge both trees into one chunk;
        # their h lives in shared hm buffers laid out [tree0 pairs | tree1
        # pairs].
        SMALL_MAX = 9
        hb = [
            [
                hpool.tile([128, 2048, 2], bf16, tag=f"h{t}0", name=f"hb{t}0"),
                hpool.tile([128, 4096, 2], bf16, tag=f"h{t}1", name=f"hb{t}1"),
            ]
            for t in range(TPC)
        ]
        hm = [
            hpool.tile([128, 1024, 2], bf16, tag="hm0", name="hm0"),
            hpool.tile([128, 512, 2], bf16, tag="hm1", name="hm1"),
        ]

        def process_chunk(C, xmov, hmov, dst, leaf):
            halves = [(q * 512, min(512, C - q * 512))
                      for q in range((C + 511) // 512)]

            def accum(ps, contribs, open_=True, close=True):
                for ci, (w, mov) in enumerate(contribs):
                    for qo, qw in halves:
                        nc.tensor.matmul(
                            ps[:, qo:qo + qw], w, mov(qo, qw),
                            start=(open_ and ci == 0),
                            stop=(close and ci == len(contribs) - 1))

            if not leaf:
                ps_r = ppool.tile([128, C], f32, tag="ps_r", name="ps_r")
                accum(ps_r, [
                    (w_ww[0][:, 0:128], lambda qo, qw: xmov(0, qo, qw)),
                    (w_ww[1][:, 0:128], lambda qo, qw: xmov(1, qo, qw)),
                    (w_ur[0][:], lambda qo, qw: hmov(0, qo, qw)),
                    (w_ur[1][:], lambda qo, qw: hmov(1, qo, qw)),
                ])
                ps_z0 = ppool.tile([128, C], f32, tag="ps_z0", name="ps_z0")
                accum(ps_z0, [
                    (w_ww[0][:, 256:384], lambda qo, qw: xmov(0, qo, qw)),
                    (w_ww[1][:, 256:384], lambda qo, qw: xmov(1, qo, qw)),
                    (w_uz[0][:, 0:128], lambda qo, qw: hmov(0, qo, qw)),
                    (w_uz[1][:, 0:128], lambda qo, qw: hmov(1, qo, qw)),
                ])
                ps_z1 = ppool.tile([128, C], f32, tag="ps_z1", name="ps_z1")
                accum(ps_z1, [
                    (w_ww[0][:, 384:512], lambda qo, qw: xmov(0, qo, qw)),
                    (w_ww[1][:, 384:512], lambda qo, qw: xmov(1, qo, qw)),
                    (w_uz[0][:, 128:256], lambda qo, qw: hmov(0, qo, qw)),
                    (w_uz[1][:, 128:256], lambda qo, qw: hmov(1, qo, qw)),
                ])
                ps_hc = ppool.tile([128, C], f32, tag="ps_hc", name="ps_hc")
                accum(ps_hc, [
                    (w_ww[0][:, 128:256], lambda qo, qw: xmov(0, qo, qw)),
                    (w_ww[1][:, 128:256], lambda qo, qw: xmov(1, qo, qw)),
                ], close=False)

                r = spool.tile([128, C], bf16, tag="r", name="r")
                nc.scalar.activation(r[:], ps_r[:], Act.Sigmoid, bias=b_r)
                rh0 = spool.tile([128, C], bf16, tag="rh0", name="rh0")
                nc.vector.tensor_mul(rh0[:], r[:], hmov(0, 0, C))
                rh1 = spool.tile([128, C], bf16, tag="rh1", name="rh1")
                nc.vector.tensor_mul(rh1[:], r[:], hmov(1, 0, C))
                accum(ps_hc, [
                    (w_uh[0][:], lambda qo, qw: rh0[:, qo:qo + qw]),
                    (w_uh[1][:], lambda qo, qw: rh1[:, qo:qo + qw]),
                ], open_=False)

                z0 = spool.tile([128, C], bf16, tag="z0", name="z0")
                nc.scalar.activation(z0[:], ps_z0[:], Act.Sigmoid, bias=b_z0)
                z1 = spool.tile([128, C], bf16, tag="z1", name="z1")
                nc.scalar.activation(z1[:], ps_z1[:], Act.Sigmoid, bias=b_z1)
                # hcn = tanh(-(ps_hc + b_hc)) = -hcand
                hcn = spool.tile([128, C], bf16, tag="hcn", name="hcn")
                nc.scalar.activation(hcn[:], ps_hc[:], Act.Tanh,
                                     bias=b_hcn, scale=-1.0)

                a = spool.tile([128, C], bf16, tag="a", name="a")
                nc.gpsimd.tensor_mul(a[:], z0[:], hmov(0, 0, C))
                bb = spool.tile([128, C], bf16, tag="bb", name="bb")
                nc.gpsimd.tensor_mul(bb[:], z1[:], hmov(1, 0, C))
                c = spool.tile([128, C], bf16, tag="c", name="c")
                nc.gpsimd.tensor_add(c[:], a[:], bb[:])
                s = spool.tile([128, C], bf16, tag="s", name="s")
                nc.vector.tensor_add(s[:], z0[:], z1[:])
                # p = (s - 1) * (-hcand) = (1 - s) * hcand
                p = spool.tile([128, C], bf16, tag="p", name="p")
                nc.vector.scalar_tensor_tensor(
                    p[:], s[:], 1.0, hcn[:], Alu.subtract, Alu.mult)
                nc.vector.tensor_add(dst, c[:], p[:])
            else:
                ps_z0 = ppool.tile([128, C], f32, tag="ps_z0", name="ps_z0")
                accum(ps_z0, [
                    (w_ww[0][:, 256:384], lambda qo, qw: xmov(0, qo, qw)),
                    (w_ww[1][:, 256:384], lambda qo, qw: xmov(1, qo, qw)),
                ])
                ps_z1 = ppool.tile([128, C], f32, tag="ps_z1", name="ps_z1")
                accum(ps_z1, [
                    (w_ww[0][:, 384:512], lambda qo, qw: xmov(0, qo, qw)),
                    (w_ww[1][:, 384:512], lambda qo, qw: xmov(1, qo, qw)),
                ])
                ps_hc = ppool.tile([128, C], f32, tag="ps_hc", name="ps_hc")
                accum(ps_hc, [
                    (w_ww[0][:, 128:256], lambda qo, qw: xmov(0, qo, qw)),
                    (w_ww[1][:, 128:256], lambda qo, qw: xmov(1, qo, qw)),
                ])
                z0 = spool.tile([128, C], bf16, tag="z0", name="z0")
                nc.scalar.activation(z0[:], ps_z0[:], Act.Sigmoid, bias=b_z0)
                z1 = spool.tile([128, C], bf16, tag="z1", name="z1")
                nc.scalar.activation(z1[:], ps_z1[:], Act.Sigmoid, bias=b_z1)
                hcn = spool.tile([128, C], bf16, tag="hcn", name="hcn")
                nc.scalar.activation(hcn[:], ps_hc[:], Act.Tanh,
                                     bias=b_hcn, scale=-1.0)
                s = spool.tile([128, C], bf16, tag="s", name="s")
                nc.gpsimd.tensor_add(s[:], z0[:], z1[:])
                nc.vector.scalar_tensor_tensor(
                    dst, s[:], 1.0, hcn[:], Alu.subtract, Alu.mult)

        for lvl in range(DEPTH, SMALL_MAX, -1):
            Fl = 2 ** lvl
            leaf = lvl == DEPTH
            par = lvl % 2
            for t in range(TPC):
                base = t * N + (Fl - 1)
                xs = []
                for s0 in range(0, Fl, SLAB):
                    W = min(SLAB, Fl - s0)
                    xt0 = xpool.tile([128, W], bf16, tag="x0", name="x0")
                    nc.sync.dma_start(xt0[:], xt[0:128, base + s0:base + s0 + W])
                    xt1 = xpool.tile([128, W], bf16, tag="x1", name="x1")
                    nc.sync.dma_start(xt1[:], xt[128:256, base + s0:base + s0 + W])
                    xs.append((xt0, xt1))

                hsrc = None if leaf else hb[t][(lvl + 1) % 2]

                for j0 in range(0, Fl, CH):
                    C = min(CH, Fl - j0)
                    xk = xs[j0 // SLAB]
                    xo = j0 % SLAB

                    def xmov(k, qo, qw, xk=xk, xo=xo):
                        return xk[k][:, xo + qo:xo + qo + qw]

                    def hmov(side, qo, qw, hsrc=hsrc, j0=j0):
                        return hsrc[:, j0 + qo:j0 + qo + qw, side]

                    if lvl == SMALL_MAX + 1:
                        dst = hm[par][:, t * (Fl // 2) + j0 // 2:
                                      t * (Fl // 2) + (j0 + C) // 2, :]
                    else:
                        dst = hb[t][par][:, j0 // 2:(j0 + C) // 2, :]
                    process_chunk(C, xmov, None if leaf else hmov, dst, leaf)

                if lvl == SMALL_MAX + 1:
                    nc.sync.dma_start(
                        h_out[:, base:base + Fl],
                        hm[par][:, t * (Fl // 2):(t + 1) * (Fl // 2), :])
                else:
                    nc.sync.dma_start(h_out[:, base:base + Fl],
                                      hb[t][par][:, 0:Fl // 2, :])

        for lvl in range(SMALL_MAX, -1, -1):
            Fl = 2 ** lvl
            par = lvl % 2
            C = 2 * Fl
            xm = []
            for k in range(2):
                xmk = xpool.tile([128, C], bf16, tag=f"xm{k}", name=f"xm{k}")
                for t in range(TPC):
                    base = t * N + (Fl - 1)
                    nc.sync.dma_start(xmk[:, t * Fl:(t + 1) * Fl],
                                      xt[k * 128:(k + 1) * 128, base:base + Fl])
                xm.append(xmk)
            hsrc = hm[(lvl + 1) % 2]

            def xmov(k, qo, qw, xm=xm):
                return xm[k][:, qo:qo + qw]

            def hmov(side, qo, qw, hsrc=hsrc):
                return hsrc[:, qo:qo + qw, side]

            if lvl > 0:
                dst = hm[par][:, 0:Fl, :]
            else:
                rt = spool.tile([128, 2], bf16, tag="root", name="rt")
                dst = rt[:]
            process_chunk(C, xmov, hmov, dst, False)

            for t in range(TPC):
                base = t * N + (Fl - 1)
                if lvl > 0:
                    nc.sync.dma_start(
                        h_out[:, base:base + Fl],
                        hm[par][:, t * (Fl // 2):(t + 1) * (Fl // 2), :])
                else:
                    nc.sync.dma_start(h_out[:, base:base + 1], rt[:, t:t + 1])

    nc.finalize()
    return nc


def _get_nc():
    global _cached
    if _cached is None:
        _cached = _build()
    return _cached


def kernel(**inputs):
    x = np.asarray(inputs["x"], dtype=np.float32)
    W_w = np.asarray(inputs["W_w"], dtype=np.float32)
    W_b = np.asarray(inputs["W_b"], dtype=np.float32)
    U_r = np.asarray(inputs["U_r"], dtype=np.float32)
    U_h = np.asarray(inputs["U_h"], dtype=np.float32)
    U_z = np.asarray(inputs["U_z"], dtype=np.float32)

    from concourse.bass_utils import run_bass_kernel_spmd

    nc = _get_nc()

    xb = x.astype(BF16)
    wwt = np.ascontiguousarray(W_w.T).reshape(2, 128, 512).astype(BF16)
    urt = np.ascontiguousarray(U_r.T).reshape(2, 128, 128).astype(BF16)
    uht = np.ascontiguousarray(U_h.T).reshape(2, 128, 128).astype(BF16)
    uzt = np.ascontiguousarray(U_z.T).reshape(2, 128, 256).astype(BF16)
    bias = np.stack(
        [W_b[:128], W_b[256:384], W_b[384:512], -W_b[128:256]], axis=1
    ).astype(np.float32)

    in_maps = []
    for c in range(NCORES):
        xt_c = np.ascontiguousarray(xb[c * NPC:(c + 1) * NPC].T)
        in_maps.append({
            "xt": xt_c, "wwt": wwt, "urt": urt, "uht": uht,
            "uzt": uzt, "bias": bias,
        })

    res = run_bass_kernel_spmd(nc, in_maps, list(range(NCORES)), **RUN_KW)
    global LAST, LAST_IN_MAPS
    LAST = res
    LAST_IN_MAPS = in_maps
    h = np.concatenate(
        [np.asarray(r["h_out"]).T for r in res.results], axis=0
    ).astype(np.float32)
    return h
